# revision 20
# baseline (speedup 1.0000x reference)
"""Trainium2 Bass kernel for nn_InvariantAttnPool.

Reference computation (per batch b):
    s      = mean_c h_v[b,c,l]                      # [L]
    logits = h_v * s * (<wq,wk>/sqrt(64))           # [C, L]
    alpha  = softmax_c(logits)
    pooled = sum_c alpha * h_v                      # [L]
    psi    = einsum("la,da->dl", pooled[:,None]*wv, w_out)

Key algebraic collapses:
  * psi[b,d,l] = pooled[b,l] * u[d] with u = w_out @ wv (host-side tiny
    contraction), so the [B,512,L] output is a rank-1 outer product per batch.
  * logits are tiny (|x| <~ 0.1), so db = sum_c exp(x) = 256*(1+eps) with
    |eps| <~ 0.03; 1/db is computed as the affine 2/256 - db/65536 (first-order
    Newton at 1/256, relative error eps^2, ~1e-5 typical), which runs as a
    fused scale+bias Copy on the Scalar engine instead of a DVE reciprocal.

Dtype strategy: HBM traffic is the roofline (~358 GB/s/core), so both boundary
tensors are fp16 on the wire: h is cast f32->fp16 on host before upload (the
kernel always computed in fp16 anyway; the cast used to happen inside the
DMA), and psi is written fp16 by the device and upcast to f32 on host during
the gather. 24 MiB/core vs the baseline's 48 MiB/core; ~3e-4 added relative
error against a 2e-2 budget.

Device pipeline per (batch, W-column chunk of L), channels as 2x128 partition
blocks packed in one [128, 2W] fp16 tile (cb0 at [0:W], cb1 at [W:2W]); all
matmuls use an all-ones [128,128] fp16 lhsT which reduces over channels and
broadcasts to all 128 partitions:
    PE : S_g  = ones.T @ h                       (channel sum)
    ACT: sq   = S_g * qs        (Copy w/ scale, PSUM->SBUF fp16)
    DVE: lg   = h * sq          (fp16 2x)
    ACT: e    = exp(lg)         (in place)
    DVE: w    = e * h           (fp16 2x)
    PE : D_q  = ones.T @ e ; N_q = ones.T @ w
    ACT: rdb  = 2/256 - D_q/65536   (Copy w/ scale+bias = 1/db)
    DVE: pb   = N_q * rdb       (PSUM 1x; for half the q's the N tile is
         first copied PSUM->SBUF fp16 on ACT so the multiply runs 2x -
         balances DVE vs ACT busy)
    DVE: ot_k = pb * u[128k:128(k+1)]   (fp16 4x tensor_scalar)
    DMA: ot_k -> psi[b, 128k:128(k+1), chunk]  (fp16)

The first/last chunks of the core's work are 512/1024 wide to shorten the
pipeline fill and the final output-DMA drain; interior chunks are 2048.

Sharding: pure data parallel over batch B=16 -> 2 batches per core x 8 cores.
"""

import math

import numpy as np

import concourse.bacc as bacc
import concourse.mybir as mybir
from concourse import tile
from concourse.bass_utils import run_bass_kernel_spmd

B, C, L = 16, 256, 8192
D_INNER, ATT_DIM = 512, 64
N_CORES = 8
BPC = B // N_CORES  # batches per core
CHUNK = 2048  # max l-columns per chunk
F32 = mybir.dt.float32
F16 = mybir.dt.float16
AF = mybir.ActivationFunctionType

# 1/db = 2/256 - db/65536 (Newton step at 1/256; db = 256*(1+eps), err=eps^2)
RDB_SCALE = -1.0 / 65536.0
RDB_BIAS = 2.0 / 256.0

# (batch, l0, width) schedule: narrow chunks at the very start (shorter
# pipeline fill) and very end (earlier final output DMAs).
def _chunk_schedule():
    sched = []
    widths0 = [512, 512, 1024, 2048, 2048, 2048]
    l0 = 0
    for w in widths0:
        sched.append((0, l0, w))
        l0 += w
    widths1 = [2048, 2048, 2048, 1024, 512, 512]
    l0 = 0
    for w in widths1:
        sched.append((1, l0, w))
        l0 += w
    return sched


_CACHE = {}


def build_nc():
    nc = bacc.Bacc(
        "TRN2",
        target_bir_lowering=False,
        debug=False,
        num_devices=N_CORES,
    )
    # channels pre-split into 2 blocks of 128 (cb, p) for single-DMA loads
    h = nc.dram_tensor("h", [BPC, 2, 128, L], F16, kind="ExternalInput")
    # u_cols[p, k] = (w_out @ wv)[128*k + p]; qs = scalar qk/2048 replicated
    u_cols = nc.dram_tensor("u_cols", [128, 4], F32, kind="ExternalInput")
    qs_arr = nc.dram_tensor("qs", [128, 1], F32, kind="ExternalInput")
    o = nc.dram_tensor("o", [BPC, D_INNER, L], F16, kind="ExternalOutput")

    with tile.TileContext(nc) as tc:
        with (
            tc.tile_pool(name="const", bufs=1) as cpool,
            tc.tile_pool(name="hin", bufs=4) as hpool,
            tc.tile_pool(name="sq", bufs=3) as sqpool,
            tc.tile_pool(name="lg", bufs=3) as lgpool,
            tc.tile_pool(name="wt", bufs=3) as wpool,
            tc.tile_pool(name="rd", bufs=3) as rpool,
            tc.tile_pool(name="nbc", bufs=3) as npool,
            tc.tile_pool(name="pool", bufs=3) as ppool,
            tc.tile_pool(name="outp", bufs=4) as opool,
            tc.tile_pool(name="ps_s", bufs=2, space="PSUM") as ps_s,
            tc.tile_pool(name="ps_d", bufs=2, space="PSUM") as ps_d,
            tc.tile_pool(name="ps_n", bufs=2, space="PSUM") as ps_n,
        ):
            ones_t = cpool.tile([128, 128], F16)
            u_t = cpool.tile([128, 4], F32)
            qs_t = cpool.tile([128, 1], F32)
            nc.vector.memset(ones_t[:], 1.0)
            nc.sync.dma_start(u_t[:], u_cols[:])
            nc.sync.dma_start(qs_t[:], qs_arr[:])

            def mm(out_ap, rhs_ap, start, stop):
                nc.tensor.matmul(out_ap, ones_t[:], rhs_ap, start=start, stop=stop)

            def head(b, l0, W):
                """Load + channel-sum + sq for one chunk."""
                # single load per chunk: SBUF side is a plain contiguous
                # write; only the DRAM side carries the (c p l -> p c l)
                # reshaped access pattern
                ht = hpool.tile([128, 2 * CHUNK], F16, tag="h")
                nc.sync.dma_start(
                    ht[:, 0 : 2 * W],
                    h[b, :, :, l0 : l0 + W].rearrange("c p l -> p c l"),
                )
                # channel sum -> sq = qs * sum_c h, fp16 broadcast on SBUF
                sq = sqpool.tile([128, CHUNK], F16, tag="sq")
                ws = min(W, 1024)
                for g in range(W // ws):
                    g0 = ws * g
                    S = ps_s.tile([128, 1024], F32, tag="S")
                    for q in range(ws // 512):
                        s0 = g0 + 512 * q
                        dst = S[:, 512 * q : 512 * (q + 1)]
                        mm(dst, ht[:, s0 : s0 + 512], True, False)
                        mm(dst, ht[:, W + s0 : W + s0 + 512], False, True)
                    nc.scalar.activation(
                        sq[:, g0 : g0 + ws], S[:, 0:ws], AF.Copy,
                        bias=0.0, scale=qs_t[:, 0:1],
                    )
                return ht, sq

            def tail(b, l0, W, ht, sq):
                """softmax + pool + output for one chunk."""
                # logits = h * sq, then e = exp(logits) in place
                lg = lgpool.tile([128, 2 * CHUNK], F16, tag="lg")
                wt = wpool.tile([128, 2 * CHUNK], F16, tag="w")
                for cb in range(2):
                    cs = slice(W * cb, W * (cb + 1))
                    nc.vector.tensor_mul(lg[:, cs], ht[:, cs], sq[:, 0:W])
                # exp and w = e * h split per channel block so the D/N
                # matmuls and the DVE multiply can start on block 0 while
                # the Scalar engine still works on block 1
                for cb in range(2):
                    cs = slice(W * cb, W * (cb + 1))
                    nc.scalar.activation(lg[:, cs], lg[:, cs], AF.Exp, bias=0.0)
                    nc.vector.tensor_mul(wt[:, cs], lg[:, cs], ht[:, cs])

                # denominator/numerator sums; pooled pb = N * (1/db)
                rdb = rpool.tile([128, CHUNK], F16, tag="rdb")
                pb = ppool.tile([128, CHUNK], F16, tag="pb")
                nq = W // 512
                for q in range(nq):
                    s0 = 512 * q
                    sl = slice(s0, s0 + 512)
                    D = ps_d.tile([128, 512], F32, tag="D")
                    mm(D[:], lg[:, sl], True, False)
                    mm(D[:], lg[:, W + s0 : W + s0 + 512], False, True)
                    N = ps_n.tile([128, 512], F32, tag="N")
                    mm(N[:], wt[:, sl], True, False)
                    mm(N[:], wt[:, W + s0 : W + s0 + 512], False, True)
                    nc.scalar.activation(
                        rdb[:, sl], D[:], AF.Copy, bias=RDB_BIAS, scale=RDB_SCALE
                    )
                    if 2 * q >= nq:
                        # ACT-side PSUM escape so the multiply runs fp16 2x on
                        # DVE (balances DVE vs ACT busy time)
                        nbs = npool.tile([128, 512], F16, tag="nbs")
                        nc.scalar.activation(nbs[:], N[:], AF.Copy, bias=0.0)
                        nc.vector.tensor_mul(pb[:, sl], nbs[:], rdb[:, sl])
                    else:
                        nc.vector.tensor_mul(pb[:, sl], N[:], rdb[:, sl])

                # psi[128k+p, l] = pb * u[128k+p], fp16 4x tensor_scalar;
                # all four k-blocks go out in one DMA (SBUF side is a plain
                # contiguous read; DRAM side carries the reshaped pattern)
                ot = opool.tile([128, 4 * CHUNK], F16, tag="ot")
                for k in range(4):
                    nc.vector.tensor_scalar_mul(
                        ot[:, W * k : W * (k + 1)], pb[:, 0:W], u_t[:, k : k + 1]
                    )
                nc.sync.dma_start(
                    o[b, :, l0 : l0 + W].rearrange("(k p) l -> p k l", k=4),
                    ot[:, 0 : 4 * W],
                )

            for c in _chunk_schedule():
                ht, sq = head(*c)
                tail(*c, ht, sq)

    nc.compile()
    return nc


def make_in_maps(h_v, wq, wk, wv, w_out):
    h16 = np.ascontiguousarray(h_v, dtype=np.float16)
    qk = np.float32(np.dot(wq.astype(np.float32), wk.astype(np.float32)))
    u = (w_out.astype(np.float32) @ wv.astype(np.float32)).astype(np.float32)
    qs = np.float32(qk / (math.sqrt(ATT_DIM) * C))

    u_cols = np.ascontiguousarray(u.reshape(4, 128).T)  # [128, 4]
    qs_arr = np.full((128, 1), qs, np.float32)

    return [
        {
            "h": np.ascontiguousarray(h16[c * BPC : (c + 1) * BPC]).reshape(
                BPC, 2, 128, L
            ),
            "u_cols": u_cols,
            "qs": qs_arr,
        }
        for c in range(N_CORES)
    ]


def gather(outs):
    return np.concatenate(outs, axis=0).astype(np.float32)


def kernel(h_v, wq, wk, wv, w_out):
    if "nc" not in _CACHE:
        _CACHE["nc"] = build_nc()
    nc = _CACHE["nc"]
    in_maps = make_in_maps(h_v, wq, wk, wv, w_out)
    res = run_bass_kernel_spmd(nc, in_maps, core_ids=list(range(N_CORES)))
    return gather([r["o"] for r in res.results])


# revision 21
# speedup vs baseline: 1.0284x; 1.0284x over previous
"""Trainium2 Bass kernel for nn_InvariantAttnPool.

Reference computation (per batch b):
    s      = mean_c h_v[b,c,l]                      # [L]
    logits = h_v * s * (<wq,wk>/sqrt(64))           # [C, L]
    alpha  = softmax_c(logits)
    pooled = sum_c alpha * h_v                      # [L]
    psi    = einsum("la,da->dl", pooled[:,None]*wv, w_out)

Key algebraic collapses:
  * psi[b,d,l] = pooled[b,l] * u[d] with u = w_out @ wv (host-side tiny
    contraction), so the [B,512,L] output is a rank-1 outer product per batch.
  * logits are tiny (|x| <~ 0.1), so db = sum_c exp(x) = 256*(1+eps) with
    |eps| <~ 0.03; 1/db is computed as the affine 2/256 - db/65536 (first-order
    Newton at 1/256, relative error eps^2, ~1e-5 typical), which runs as a
    fused scale+bias Copy on the Scalar engine instead of a DVE reciprocal.

Dtype strategy: HBM traffic is the roofline (~358 GB/s/core), so both boundary
tensors are fp16 on the wire: h is cast f32->fp16 on host before upload (the
kernel always computed in fp16 anyway; the cast used to happen inside the
DMA), and psi is written fp16 by the device and upcast to f32 on host during
the gather. 24 MiB/core vs the baseline's 48 MiB/core; ~3e-4 added relative
error against a 2e-2 budget.

Device pipeline per (batch, W-column chunk of L), channels as 2x128 partition
blocks packed in one [128, 2W] fp16 tile (cb0 at [0:W], cb1 at [W:2W]); all
matmuls use an all-ones [128,128] fp16 lhsT which reduces over channels and
broadcasts to all 128 partitions:
    PE : S_g  = ones.T @ h                       (channel sum)
    ACT: sq   = S_g * qs        (Copy w/ scale, PSUM->SBUF fp16)
    DVE: lg   = h * sq          (fp16 2x)
    ACT: e    = exp(lg)         (in place)
    DVE: w    = e * h           (fp16 2x)
    PE : D_q  = ones.T @ e ; N_q = ones.T @ w
    ACT: rdb  = 2/256 - D_q/65536   (Copy w/ scale+bias = 1/db)
    DVE: pb   = N_q * rdb       (PSUM 1x; for half the q's the N tile is
         first copied PSUM->SBUF fp16 on ACT so the multiply runs 2x -
         balances DVE vs ACT busy)
    DVE: ot_k = pb * u[128k:128(k+1)]   (fp16 4x tensor_scalar)
    DMA: ot_k -> psi[b, 128k:128(k+1), chunk]  (fp16)

The first/last chunks of the core's work are 512/1024 wide to shorten the
pipeline fill and the final output-DMA drain; interior chunks are 2048.

Sharding: pure data parallel over batch B=16 -> 2 batches per core x 8 cores.
"""

import math

import numpy as np

import concourse.bacc as bacc
import concourse.mybir as mybir
from concourse import tile
from concourse.bass_utils import run_bass_kernel_spmd

B, C, L = 16, 256, 8192
D_INNER, ATT_DIM = 512, 64
N_CORES = 8
BPC = B // N_CORES  # batches per core
CHUNK = 2048  # max l-columns per chunk
F32 = mybir.dt.float32
F16 = mybir.dt.float16
AF = mybir.ActivationFunctionType

# 1/db = 2/256 - db/65536 (Newton step at 1/256; db = 256*(1+eps), err=eps^2)
RDB_SCALE = -1.0 / 65536.0
RDB_BIAS = 2.0 / 256.0

# (batch, l0, width) schedule: narrow chunks at the very start (shorter
# pipeline fill) and very end (earlier final output DMAs).
def _chunk_schedule():
    sched = []
    widths0 = [512, 512, 1024, 2048, 2048, 2048]
    l0 = 0
    for w in widths0:
        sched.append((0, l0, w))
        l0 += w
    widths1 = [2048, 2048, 2048, 1024, 512, 512]
    l0 = 0
    for w in widths1:
        sched.append((1, l0, w))
        l0 += w
    return sched


_CACHE = {}


def build_nc():
    nc = bacc.Bacc(
        "TRN2",
        target_bir_lowering=False,
        debug=False,
        num_devices=N_CORES,
    )
    # channels pre-split into 2 blocks of 128 (cb, p) for single-DMA loads
    h = nc.dram_tensor("h", [BPC, 2, 128, L], F16, kind="ExternalInput")
    # u_cols[p, k] = (w_out @ wv)[128*k + p]; qs = scalar qk/2048 replicated
    u_cols = nc.dram_tensor("u_cols", [128, 4], F32, kind="ExternalInput")
    qs_arr = nc.dram_tensor("qs", [128, 1], F32, kind="ExternalInput")
    o = nc.dram_tensor("o", [BPC, D_INNER, L], F16, kind="ExternalOutput")

    with tile.TileContext(nc) as tc:
        with (
            tc.tile_pool(name="const", bufs=1) as cpool,
            tc.tile_pool(name="hin", bufs=4) as hpool,
            tc.tile_pool(name="sq", bufs=3) as sqpool,
            tc.tile_pool(name="lg", bufs=3) as lgpool,
            tc.tile_pool(name="wt", bufs=3) as wpool,
            tc.tile_pool(name="rd", bufs=3) as rpool,
            tc.tile_pool(name="nbc", bufs=3) as npool,
            tc.tile_pool(name="pool", bufs=3) as ppool,
            tc.tile_pool(name="outp", bufs=4) as opool,
            tc.tile_pool(name="ps_s", bufs=2, space="PSUM") as ps_s,
            tc.tile_pool(name="ps_d", bufs=2, space="PSUM") as ps_d,
            tc.tile_pool(name="ps_n", bufs=2, space="PSUM") as ps_n,
        ):
            ones_t = cpool.tile([128, 128], F16)
            u_t = cpool.tile([128, 4], F32)
            qs_t = cpool.tile([128, 1], F32)
            nc.vector.memset(ones_t[:], 1.0)
            nc.sync.dma_start(u_t[:], u_cols[:])
            nc.sync.dma_start(qs_t[:], qs_arr[:])

            def mm(out_ap, rhs_ap, start, stop):
                nc.tensor.matmul(out_ap, ones_t[:], rhs_ap, start=start, stop=stop)

            def head(b, l0, W):
                """Load + channel-sum + sq for one chunk."""
                # single load per chunk: SBUF side is a plain contiguous
                # write; only the DRAM side carries the (c p l -> p c l)
                # reshaped access pattern
                ht = hpool.tile([128, 2 * CHUNK], F16, tag="h")
                nc.sync.dma_start(
                    ht[:, 0 : 2 * W],
                    h[b, :, :, l0 : l0 + W].rearrange("c p l -> p c l"),
                )
                # channel sum -> sq = qs * sum_c h, fp16 broadcast on SBUF
                sq = sqpool.tile([128, CHUNK], F16, tag="sq")
                ws = min(W, 1024)
                for g in range(W // ws):
                    g0 = ws * g
                    S = ps_s.tile([128, 1024], F32, tag="S")
                    for q in range(ws // 512):
                        s0 = g0 + 512 * q
                        dst = S[:, 512 * q : 512 * (q + 1)]
                        mm(dst, ht[:, s0 : s0 + 512], True, False)
                        mm(dst, ht[:, W + s0 : W + s0 + 512], False, True)
                    nc.scalar.activation(
                        sq[:, g0 : g0 + ws], S[:, 0:ws], AF.Copy,
                        bias=0.0, scale=qs_t[:, 0:1],
                    )
                return ht, sq

            def tail(b, l0, W, ht, sq):
                """softmax + pool + output for one chunk."""
                # logits = h * sq, then e = exp(logits) in place
                lg = lgpool.tile([128, 2 * CHUNK], F16, tag="lg")
                for cb in range(2):
                    cs = slice(W * cb, W * (cb + 1))
                    nc.vector.tensor_mul(lg[:, cs], ht[:, cs], sq[:, 0:W])
                nc.scalar.activation(lg[:, 0 : 2 * W], lg[:, 0 : 2 * W], AF.Exp, bias=0.0)

                # w = e * h
                wt = wpool.tile([128, 2 * CHUNK], F16, tag="w")
                nc.vector.tensor_mul(wt[:, 0 : 2 * W], lg[:, 0 : 2 * W], ht[:, 0 : 2 * W])

                # denominator/numerator sums; pooled pb = N * (1/db)
                rdb = rpool.tile([128, CHUNK], F16, tag="rdb")
                pb = ppool.tile([128, CHUNK], F16, tag="pb")
                nq = W // 512
                for q in range(nq):
                    s0 = 512 * q
                    sl = slice(s0, s0 + 512)
                    D = ps_d.tile([128, 512], F32, tag="D")
                    mm(D[:], lg[:, sl], True, False)
                    mm(D[:], lg[:, W + s0 : W + s0 + 512], False, True)
                    N = ps_n.tile([128, 512], F32, tag="N")
                    mm(N[:], wt[:, sl], True, False)
                    mm(N[:], wt[:, W + s0 : W + s0 + 512], False, True)
                    nc.scalar.activation(
                        rdb[:, sl], D[:], AF.Copy, bias=RDB_BIAS, scale=RDB_SCALE
                    )
                    if 2 * q >= nq:
                        # ACT-side PSUM escape so the multiply runs fp16 2x on
                        # DVE (balances DVE vs ACT busy time)
                        nbs = npool.tile([128, 512], F16, tag="nbs")
                        nc.scalar.activation(nbs[:], N[:], AF.Copy, bias=0.0)
                        nc.vector.tensor_mul(pb[:, sl], nbs[:], rdb[:, sl])
                    else:
                        nc.vector.tensor_mul(pb[:, sl], N[:], rdb[:, sl])

                # psi[128k+p, l] = pb * u[128k+p], fp16 4x tensor_scalar;
                # all four k-blocks go out in one DMA (SBUF side is a plain
                # contiguous read; DRAM side carries the reshaped pattern)
                ot = opool.tile([128, 4 * CHUNK], F16, tag="ot")
                for k in range(4):
                    nc.vector.tensor_scalar_mul(
                        ot[:, W * k : W * (k + 1)], pb[:, 0:W], u_t[:, k : k + 1]
                    )
                nc.sync.dma_start(
                    o[b, :, l0 : l0 + W].rearrange("(k p) l -> p k l", k=4),
                    ot[:, 0 : 4 * W],
                )

            for c in _chunk_schedule():
                ht, sq = head(*c)
                tail(*c, ht, sq)

    nc.compile()
    return nc


def make_in_maps(h_v, wq, wk, wv, w_out):
    h16 = np.ascontiguousarray(h_v, dtype=np.float16)
    qk = np.float32(np.dot(wq.astype(np.float32), wk.astype(np.float32)))
    u = (w_out.astype(np.float32) @ wv.astype(np.float32)).astype(np.float32)
    qs = np.float32(qk / (math.sqrt(ATT_DIM) * C))

    u_cols = np.ascontiguousarray(u.reshape(4, 128).T)  # [128, 4]
    qs_arr = np.full((128, 1), qs, np.float32)

    return [
        {
            "h": np.ascontiguousarray(h16[c * BPC : (c + 1) * BPC]).reshape(
                BPC, 2, 128, L
            ),
            "u_cols": u_cols,
            "qs": qs_arr,
        }
        for c in range(N_CORES)
    ]


def gather(outs):
    return np.concatenate(outs, axis=0).astype(np.float32)


def kernel(h_v, wq, wk, wv, w_out):
    if "nc" not in _CACHE:
        _CACHE["nc"] = build_nc()
    nc = _CACHE["nc"]
    in_maps = make_in_maps(h_v, wq, wk, wv, w_out)
    res = run_bass_kernel_spmd(nc, in_maps, core_ids=list(range(N_CORES)))
    return gather([r["o"] for r in res.results])


# revision 23
# speedup vs baseline: 1.0621x; 1.0328x over previous
"""Trainium2 Bass kernel for nn_InvariantAttnPool.

Reference computation (per batch b):
    s      = mean_c h_v[b,c,l]                      # [L]
    logits = h_v * s * (<wq,wk>/sqrt(64))           # [C, L]
    alpha  = softmax_c(logits)
    pooled = sum_c alpha * h_v                      # [L]
    psi    = einsum("la,da->dl", pooled[:,None]*wv, w_out)

Key algebraic collapses:
  * psi[b,d,l] = pooled[b,l] * u[d] with u = w_out @ wv (host-side tiny
    contraction), so the [B,512,L] output is a rank-1 outer product per batch.
  * logits are tiny (|x| <~ 0.1), so db = sum_c exp(x) = 256*(1+eps) with
    |eps| <~ 0.03; 1/db is computed as the affine 2/256 - db/65536 (first-order
    Newton at 1/256, relative error eps^2, ~1e-5 typical), which runs as a
    fused scale+bias Copy on the Scalar engine instead of a DVE reciprocal.

Dtype strategy: HBM traffic is the roofline (~358 GB/s/core), so both boundary
tensors are fp16 on the wire: h is cast f32->fp16 on host before upload (the
kernel always computed in fp16 anyway; the cast used to happen inside the
DMA), and psi is written fp16 by the device and upcast to f32 on host during
the gather. 24 MiB/core vs the baseline's 48 MiB/core; ~3e-4 added relative
error against a 2e-2 budget.

Device pipeline per (batch, W-column chunk of L), channels as 2x128 partition
blocks packed in one [128, 2W] fp16 tile (cb0 at [0:W], cb1 at [W:2W]); all
matmuls use an all-ones [128,128] fp16 lhsT which reduces over channels and
broadcasts to all 128 partitions:
    PE : S_g  = ones.T @ h                       (channel sum)
    ACT: sq   = S_g * qs        (Copy w/ scale, PSUM->SBUF fp16)
    DVE: lg   = h * sq          (fp16 2x)
    ACT: e    = exp(lg)         (in place)
    DVE: w    = e * h           (fp16 2x)
    PE : D_q  = ones.T @ e ; N_q = ones.T @ w
    ACT: rdb  = 2/256 - D_q/65536   (Copy w/ scale+bias = 1/db)
    DVE: pb   = N_q * rdb       (PSUM 1x; for half the q's the N tile is
         first copied PSUM->SBUF fp16 on ACT so the multiply runs 2x -
         balances DVE vs ACT busy)
    DVE: ot_k = pb * u[128k:128(k+1)]   (fp16 4x tensor_scalar)
    DMA: ot_k -> psi[b, 128k:128(k+1), chunk]  (fp16)

The first/last chunks of the core's work are 512/1024 wide to shorten the
pipeline fill and the final output-DMA drain; interior chunks are 2048.

Sharding: pure data parallel over batch B=16 -> 2 batches per core x 8 cores.
"""

import math

import numpy as np

import concourse.bacc as bacc
import concourse.mybir as mybir
from concourse import tile
from concourse.bass_utils import run_bass_kernel_spmd

B, C, L = 16, 256, 8192
D_INNER, ATT_DIM = 512, 64
N_CORES = 8
BPC = B // N_CORES  # batches per core
CHUNK = 2048  # max l-columns per chunk
F32 = mybir.dt.float32
F16 = mybir.dt.float16
AF = mybir.ActivationFunctionType

# 1/db = 2/256 - db/65536 (Newton step at 1/256; db = 256*(1+eps), err=eps^2)
RDB_SCALE = -1.0 / 65536.0
RDB_BIAS = 2.0 / 256.0

# (batch, l0, width) schedule: narrow chunks at the very start (shorter
# pipeline fill) and very end (earlier final output DMAs).
def _chunk_schedule():
    sched = []
    widths0 = [512, 512, 1024, 2048, 2048, 2048]
    l0 = 0
    for w in widths0:
        sched.append((0, l0, w))
        l0 += w
    widths1 = [2048, 2048, 2048, 1024, 512, 512]
    l0 = 0
    for w in widths1:
        sched.append((1, l0, w))
        l0 += w
    return sched


_CACHE = {}


def build_nc():
    nc = bacc.Bacc(
        "TRN2",
        target_bir_lowering=False,
        debug=False,
        num_devices=N_CORES,
    )
    # channels pre-split into 2 blocks of 128 (cb, p) for single-DMA loads
    h = nc.dram_tensor("h", [BPC, 2, 128, L], F16, kind="ExternalInput")
    # u_cols[p, k] = (w_out @ wv)[128*k + p]; qs = scalar qk/2048 replicated
    u_cols = nc.dram_tensor("u_cols", [128, 4], F32, kind="ExternalInput")
    qs_arr = nc.dram_tensor("qs", [128, 1], F32, kind="ExternalInput")
    o = nc.dram_tensor("o", [BPC, D_INNER, L], F16, kind="ExternalOutput")

    with tile.TileContext(nc) as tc:
        with (
            tc.tile_pool(name="const", bufs=1) as cpool,
            tc.tile_pool(name="hin", bufs=5) as hpool,
            tc.tile_pool(name="sq", bufs=3) as sqpool,
            tc.tile_pool(name="lg", bufs=3) as lgpool,
            tc.tile_pool(name="wt", bufs=3) as wpool,
            tc.tile_pool(name="rd", bufs=3) as rpool,
            tc.tile_pool(name="nbc", bufs=3) as npool,
            tc.tile_pool(name="pool", bufs=3) as ppool,
            tc.tile_pool(name="outp", bufs=4) as opool,
            tc.tile_pool(name="ps_s", bufs=2, space="PSUM") as ps_s,
            tc.tile_pool(name="ps_d", bufs=2, space="PSUM") as ps_d,
            tc.tile_pool(name="ps_n", bufs=2, space="PSUM") as ps_n,
        ):
            ones_t = cpool.tile([128, 128], F16)
            u_t = cpool.tile([128, 4], F32)
            qs_t = cpool.tile([128, 1], F32)
            warm_t = cpool.tile([128, 512], F16)
            nc.vector.memset(ones_t[:], 1.0)
            nc.vector.memset(warm_t[:], 0.0)
            nc.sync.dma_start(u_t[:], u_cols[:])
            nc.sync.dma_start(qs_t[:], qs_arr[:])

            def mm(out_ap, rhs_ap, start, stop):
                nc.tensor.matmul(out_ap, ones_t[:], rhs_ap, start=start, stop=stop)

            # Warm-up while the first input DMA is in flight: ~10 throwaway
            # matmuls ramp the PE out of its cold p-state (~2x clock), and a
            # dummy exp pulls the ACT_TABLE_LOAD (~1.3us) off chunk 0's
            # critical path.
            nc.scalar.activation(warm_t[:, 0:16], warm_t[:, 0:16], AF.Exp, bias=0.0)
            for i in range(10):
                wp = ps_d.tile([128, 512], F32, tag="D")
                mm(wp[:], warm_t[:], True, True)

            def head(b, l0, W):
                """Load + channel-sum + sq for one chunk."""
                # single load per chunk: SBUF side is a plain contiguous
                # write; only the DRAM side carries the (c p l -> p c l)
                # reshaped access pattern
                ht = hpool.tile([128, 2 * CHUNK], F16, tag="h")
                nc.sync.dma_start(
                    ht[:, 0 : 2 * W],
                    h[b, :, :, l0 : l0 + W].rearrange("c p l -> p c l"),
                )
                # channel sum -> sq = qs * sum_c h, fp16 broadcast on SBUF
                sq = sqpool.tile([128, CHUNK], F16, tag="sq")
                ws = min(W, 1024)
                for g in range(W // ws):
                    g0 = ws * g
                    S = ps_s.tile([128, 1024], F32, tag="S")
                    for q in range(ws // 512):
                        s0 = g0 + 512 * q
                        dst = S[:, 512 * q : 512 * (q + 1)]
                        mm(dst, ht[:, s0 : s0 + 512], True, False)
                        mm(dst, ht[:, W + s0 : W + s0 + 512], False, True)
                    nc.scalar.activation(
                        sq[:, g0 : g0 + ws], S[:, 0:ws], AF.Copy,
                        bias=0.0, scale=qs_t[:, 0:1],
                    )
                return ht, sq

            def tail(b, l0, W, ht, sq):
                """softmax + pool + output for one chunk."""
                # logits = h * sq, then e = exp(logits) in place
                lg = lgpool.tile([128, 2 * CHUNK], F16, tag="lg")
                for cb in range(2):
                    cs = slice(W * cb, W * (cb + 1))
                    nc.vector.tensor_mul(lg[:, cs], ht[:, cs], sq[:, 0:W])
                nc.scalar.activation(lg[:, 0 : 2 * W], lg[:, 0 : 2 * W], AF.Exp, bias=0.0)

                # w = e * h
                wt = wpool.tile([128, 2 * CHUNK], F16, tag="w")
                nc.vector.tensor_mul(wt[:, 0 : 2 * W], lg[:, 0 : 2 * W], ht[:, 0 : 2 * W])

                # denominator/numerator sums; pooled pb = N * (1/db)
                rdb = rpool.tile([128, CHUNK], F16, tag="rdb")
                pb = ppool.tile([128, CHUNK], F16, tag="pb")
                nq = W // 512
                for q in range(nq):
                    s0 = 512 * q
                    sl = slice(s0, s0 + 512)
                    D = ps_d.tile([128, 512], F32, tag="D")
                    mm(D[:], lg[:, sl], True, False)
                    mm(D[:], lg[:, W + s0 : W + s0 + 512], False, True)
                    N = ps_n.tile([128, 512], F32, tag="N")
                    mm(N[:], wt[:, sl], True, False)
                    mm(N[:], wt[:, W + s0 : W + s0 + 512], False, True)
                    nc.scalar.activation(
                        rdb[:, sl], D[:], AF.Copy, bias=RDB_BIAS, scale=RDB_SCALE
                    )
                    if 2 * q >= nq:
                        # ACT-side PSUM escape so the multiply runs fp16 2x on
                        # DVE (balances DVE vs ACT busy time)
                        nbs = npool.tile([128, 512], F16, tag="nbs")
                        nc.scalar.activation(nbs[:], N[:], AF.Copy, bias=0.0)
                        nc.vector.tensor_mul(pb[:, sl], nbs[:], rdb[:, sl])
                    else:
                        nc.vector.tensor_mul(pb[:, sl], N[:], rdb[:, sl])

                # psi[128k+p, l] = pb * u[128k+p], fp16 4x tensor_scalar;
                # all four k-blocks go out in one DMA (SBUF side is a plain
                # contiguous read; DRAM side carries the reshaped pattern)
                ot = opool.tile([128, 4 * CHUNK], F16, tag="ot")
                for k in range(4):
                    nc.vector.tensor_scalar_mul(
                        ot[:, W * k : W * (k + 1)], pb[:, 0:W], u_t[:, k : k + 1]
                    )
                nc.sync.dma_start(
                    o[b, :, l0 : l0 + W].rearrange("(k p) l -> p k l", k=4),
                    ot[:, 0 : 4 * W],
                )

            for c in _chunk_schedule():
                ht, sq = head(*c)
                tail(*c, ht, sq)

    nc.compile()
    return nc


def make_in_maps(h_v, wq, wk, wv, w_out):
    h16 = np.ascontiguousarray(h_v, dtype=np.float16)
    qk = np.float32(np.dot(wq.astype(np.float32), wk.astype(np.float32)))
    u = (w_out.astype(np.float32) @ wv.astype(np.float32)).astype(np.float32)
    qs = np.float32(qk / (math.sqrt(ATT_DIM) * C))

    u_cols = np.ascontiguousarray(u.reshape(4, 128).T)  # [128, 4]
    qs_arr = np.full((128, 1), qs, np.float32)

    return [
        {
            "h": np.ascontiguousarray(h16[c * BPC : (c + 1) * BPC]).reshape(
                BPC, 2, 128, L
            ),
            "u_cols": u_cols,
            "qs": qs_arr,
        }
        for c in range(N_CORES)
    ]


def gather(outs):
    return np.concatenate(outs, axis=0).astype(np.float32)


def kernel(h_v, wq, wk, wv, w_out):
    if "nc" not in _CACHE:
        _CACHE["nc"] = build_nc()
    nc = _CACHE["nc"]
    in_maps = make_in_maps(h_v, wq, wk, wv, w_out)
    res = run_bass_kernel_spmd(nc, in_maps, core_ids=list(range(N_CORES)))
    return gather([r["o"] for r in res.results])
